# revision 35
# baseline (speedup 1.0000x reference)
"""LSTM autoencoder (B=8192, T=50, F=24; H1=64, LAT=32, H3=64) on 8 trn2 cores.

Data parallel over batch: each core handles Bc=1024 rows. Host transposes x to
[T, F, Bc] (feature-major) so all device DMAs are contiguous, and pre-packs the
LSTM weights as stationary lhsT blobs with recurrent/input weights and biases
concatenated along the contraction dim.

Phase 1 interleaves LSTM1 (step t+1) with LSTM2 (step t) in one loop so the
two recurrence chains overlap across engines. Phase 2 runs LSTM3 with the
TimeDistributed dense emitted per-step (bias folded into the matmul via a
ones-row in the rhs).

Per-LSTM-step gate math (c >= 0 always since i,f in (0,1), g = relu(.) >= 0,
so relu(c) == c and the reference's h = o*relu(c) is o*c):

    ps_fi = matmul -> sigmoid -> (f | i)           [128, Bc] psum
    ps_og = matmul -> sigmoid(o) | relu(g)->X_hi   [128, Bc] psum
    prods = sb_fi * X          (X = (c | relu_g), one [128] DVE op)
    c     = prods_lo + prods_hi  (in-place into X_lo)
    h     = sb_o * c           (writes fp16 straight into next rhs tile)
"""

import os
import sys

import numpy as np

sys.path.insert(0, "/opt/trn_rl_repo")

import concourse.bass as bass
import concourse.mybir as mybir
from concourse.bass_utils import run_bass_kernel_spmd
from concourse.tile import TileContext
from contextlib import ExitStack

B, T, F = 8192, 50, 24
H1, LAT, H3 = 64, 32, 64
NCORES = 8
Bc = B // NCORES  # 1024
HALF = Bc // 2  # max moving free dim per matmul

f16 = mybir.dt.float16
f32 = mybir.dt.float32
AF = mybir.ActivationFunctionType
Alu = mybir.AluOpType

_CACHE = {}

# ---------------------------------------------------------------------------
# Toolchain compat: the walrus build in this container predates two features
# the current Tile framework emits.
#
# 1. Tile's kernel-tail all-engine barrier uses InstEventSemaphore (the EVSEM
#    butterfly), which this walrus cannot codegen (visitInstEventSemaphore
#    throws). Replace it with the legacy 0xD5 PSEUDO_SYNC_BARRIER that NRT
#    expands at load time.
# 2. Tile attaches up to 4 semaphore waits to a single instruction;
#    setupSyncWait here handles exactly one. Split extras into single-wait
#    NoOps prepended on the same engine (engines are in-order, so waiting on
#    the nops first is equivalent).
# ---------------------------------------------------------------------------

bass.Bass.all_engine_barrier = (
    lambda self, *, sem_only=False: self._nrt_pseudo_barrier()
)
bass.Bass.multi_engine_barrier = lambda self, engines: self._nrt_pseudo_barrier()


def _split_multi_waits(js: bytes) -> bytes:
    import json

    m = json.loads(js)
    for fn in m["functions"]:
        for blk in fn["blocks"]:
            out = []
            for inst in blk["instructions"]:
                si = inst.get("sync_info")
                waits = (si or {}).get("on_wait") or []
                if len(waits) > 1:
                    for k, w in enumerate(waits[:-1]):
                        out.append(
                            {
                                "name": f"{inst['name']}_w{k}",
                                "engine": inst["engine"],
                                "opcode": "NoOp",
                                "debug": inst.get("debug", 0),
                                "ins": [],
                                "outs": [],
                                "sync_info": {"on_update": [], "on_wait": [w]},
                            }
                        )
                    si["on_wait"] = [waits[-1]]
                out.append(inst)
            blk["instructions"] = out
    return json.dumps(m).encode()


def _wrap_to_json(nc):
    orig = nc.to_json_bytes
    nc.to_json_bytes = lambda: _split_multi_waits(orig())
    return nc


def _build_nc(repeat=1):
    nc = bass.Bass()

    K1 = H1 + F + 1  # 89: rhs = [h1; x; 1]
    # rhs for LSTM3 = [h3(0:64); 1(64); zero-pad(65:96); z(96:128)] — z sits at
    # base partition 96 (DVE writes need 32-multiple partition offsets); the
    # pad rows carry zero weights so K=128 costs nothing extra.
    K3 = 128

    xT_d = nc.dram_tensor("xT", [T, F + 1, Bc], f16, kind="ExternalInput")
    w_fi1_d = nc.dram_tensor("w_fi1", [K1, 128], f16, kind="ExternalInput")
    w_og1_d = nc.dram_tensor("w_og1", [K1, 128], f16, kind="ExternalInput")
    w_u2_d = nc.dram_tensor("w_u2", [LAT + 1, 128], f16, kind="ExternalInput")
    w_w2_d = nc.dram_tensor("w_w2", [H1, 128], f16, kind="ExternalInput")
    w_fi3_d = nc.dram_tensor("w_fi3", [K3, 128], f16, kind="ExternalInput")
    w_og3_d = nc.dram_tensor("w_og3", [K3, 128], f16, kind="ExternalInput")
    w_d_d = nc.dram_tensor("w_d", [H3 + 1, F], f16, kind="ExternalInput")
    NG = (T + 2) // 3  # dense output groups of 3 steps, 32 rows each
    yT_d = nc.dram_tensor("yT", [NG, 96, Bc], f16, kind="ExternalOutput")

    halves = (slice(0, HALF), slice(HALF, Bc))

    with TileContext(nc) as tc:
     for _rep in range(repeat):
      with ExitStack() as ctx:
        wp = ctx.enter_context(tc.tile_pool(name=f"wp{_rep}", bufs=1))
        big = ctx.enter_context(tc.tile_pool(name=f"big{_rep}", bufs=1))
        sp = ctx.enter_context(tc.tile_pool(name=f"sp{_rep}", bufs=2))
        pp = ctx.enter_context(tc.tile_pool(name=f"pp{_rep}", bufs=1, space="PSUM"))
        op = ctx.enter_context(tc.tile_pool(name=f"op{_rep}", bufs=3))

        w_fi1 = wp.tile([K1, 128], f16)
        nc.sync.dma_start(out=w_fi1, in_=w_fi1_d[:])
        w_og1 = wp.tile([K1, 128], f16)
        nc.sync.dma_start(out=w_og1, in_=w_og1_d[:])
        w_u2 = wp.tile([LAT + 1, 128], f16)
        nc.sync.dma_start(out=w_u2, in_=w_u2_d[:])
        w_w2 = wp.tile([H1, 128], f16)
        nc.sync.dma_start(out=w_w2, in_=w_w2_d[:])
        w_fi3 = wp.tile([K3, 128], f16)
        nc.sync.dma_start(out=w_fi3, in_=w_fi3_d[:])
        w_og3 = wp.tile([K3, 128], f16)
        nc.sync.dma_start(out=w_og3, in_=w_og3_d[:])
        w_d = wp.tile([H3 + 1, F], f16)
        nc.sync.dma_start(out=w_d, in_=w_d_d[:])

        # ---- persistent state -------------------------------------------
        # cat1[:, t*Bc:(t+1)*Bc] = [h1_t; x_{t+1}; 1]  (block 0: h1_0 = 0)
        cat1 = big.tile([K1, (T + 1) * Bc], f16)
        nc.vector.memset(cat1[0:H1, 0:Bc], 0)
        for t in range(T):
            sl = slice(t * Bc, (t + 1) * Bc)
            nc.sync.dma_start(out=cat1[H1 : H1 + F + 1, sl], in_=xT_d[t])
        # X1 = (c1 | relu_g1): lo half persistent c1, hi half per-step relu(g)
        X1 = big.tile([128, Bc], f16)
        nc.vector.memset(X1[0:H1, :], 0)
        # cat2 ping-pong: [h2; 1]
        cat2a = big.tile([LAT + 1, Bc], f16)
        cat2b = big.tile([LAT + 1, Bc], f16)
        nc.vector.memset(cat2b[0:LAT, :], 0)  # h2_0 (read at t=1)
        nc.vector.memset(cat2a[LAT : LAT + 1, :], 1.0)
        nc.vector.memset(cat2b[LAT : LAT + 1, :], 1.0)
        c2 = big.tile([LAT, Bc], f16)
        nc.vector.memset(c2, 0)
        cat2 = (cat2a, cat2b)
        # cat3 ping-pong: [h3; 1; pad; z]
        cat3a = big.tile([K3, Bc], f16)
        cat3b = big.tile([K3, Bc], f16)
        nc.vector.memset(cat3a[0:H3, :], 0)  # h3_0
        nc.vector.memset(cat3a[H3:96, :], 1.0)  # ones row + pad (pad w = 0)
        nc.vector.memset(cat3b[H3:96, :], 1.0)
        cat3 = (cat3a, cat3b)
        X3 = big.tile([128, Bc], f16)
        nc.vector.memset(X3[0:H3, :], 0)

        # ================= phase 1: LSTM1 + LSTM2 ========================
        # Software pipeline, per iteration t:
        #   PE : og1(t), fi1(t), mm2(s=t-1) into ping-ponged pC
        #   DVE early: ig2/cadd/h2 for s=t-2 (inputs all ready last iter)
        #   ACT: relu g1(t), per-half sigmoids for LSTM1(t)
        #   DVE: LSTM1 cell(t) per half
        #   ACT end: sigmoid fio2(s=t-1), Pool: fc2(s=t-1)
        # LSTM2 is processed per batch-half (FD 512) so its loop latency fits
        # the steady-state period; nothing of LSTM2 sits ahead of LSTM1's
        # chain-critical ops on any engine.
        l2_state = {}  # s -> [ps2, (sb_fio2, fc2)]

        def lstm2_mm(s):
            cur2 = cat2[s % 2]
            h1_s = cat1[0:H1, s * Bc : (s + 1) * Bc]
            ps2 = pp.tile([128, Bc], f32, tag=f"pC{s % 2}")
            for cs in halves:
                nc.tensor.matmul(ps2[:, cs], w_u2, cur2[:, cs], start=True, stop=False)
                nc.tensor.matmul(ps2[:, cs], w_w2, h1_s[:, cs], start=False, stop=True)
            l2_state[s] = [ps2, None]

        def lstm2_sig(s):
            # ps2 gate order: (f2 | i2 | o2 | g2); per-half fc2 on Pool right
            # after each sigmoid half so it lands before next iter's cadd2.
            # o2 is copied down to base partition 0 (Pool) for the h2 mul.
            ps2 = l2_state[s][0]
            sb_fio2 = sp.tile([96, Bc], f16, tag="sb_fio2")
            fc2 = sp.tile([LAT, Bc], f16, tag="fc2")
            for cs in halves:
                nc.scalar.activation(sb_fio2[:, cs], ps2[0:96, cs], AF.Sigmoid)
                nc.gpsimd.tensor_mul(fc2[:, cs], sb_fio2[0:LAT, cs], c2[:, cs])
            o2t = sp.tile([LAT, Bc], f16, tag="o2t")
            nc.gpsimd.tensor_copy(o2t, sb_fio2[2 * LAT : 3 * LAT])
            l2_state[s][1] = (sb_fio2, fc2, o2t)

        def lstm2_cell(s, z_dests=None):
            ps2, (sb_fio2, fc2, o2t) = l2_state.pop(s)
            ig2 = sp.tile([LAT, Bc], f16, tag="ig2")
            for cs in halves:
                nc.vector.scalar_tensor_tensor(
                    ig2[:, cs], ps2[96:128, cs], 0.0, sb_fio2[LAT : 2 * LAT, cs],
                    Alu.max, Alu.mult,
                )
                nc.vector.tensor_add(c2[:, cs], fc2[:, cs], ig2[:, cs])
                if z_dests is None:
                    nxt2 = cat2[(s + 1) % 2]
                    nc.vector.tensor_mul(nxt2[0:LAT, cs], o2t[:, cs], c2[:, cs])
            if z_dests is not None:
                for zd in z_dests:
                    nc.vector.tensor_mul(zd, o2t, c2)

        for t in range(T + 3):
            s_mm, s_cell = t - 1, t - 2
            if t < T:
                base = t * Bc
                rhs1 = cat1[:, base : base + Bc]
                ps_og = pp.tile([128, Bc], f32, tag="pB")
                for cs in halves:
                    nc.tensor.matmul(ps_og[:, cs], w_og1, rhs1[:, cs], start=True, stop=True)
                ps_fi = pp.tile([128, Bc], f32, tag="pA")
                for cs in halves:
                    nc.tensor.matmul(ps_fi[:, cs], w_fi1, rhs1[:, cs], start=True, stop=True)
            if 1 <= s_cell <= T:
                zd = [cat3a[96:K3], cat3b[96:K3]] if s_cell == T else None
                lstm2_cell(s_cell, z_dests=zd)  # early DVE slot; writes h2(s_cell)
            if 1 <= s_mm <= T:
                lstm2_mm(s_mm)  # consumes h2(s_mm - 1) written just above
            if t < T:
                # ACT: relu g -> X1 hi, then per-half sigmoid (f|i) / sigmoid o
                nc.scalar.activation(X1[H1:128], ps_og[H1:128], AF.Relu)
                sb_fi = sp.tile([128, Bc], f16, tag="sb_fi")
                sb_o = sp.tile([H1, Bc], f16, tag="sb_o")
                prods = sp.tile([128, Bc], f16, tag="prods")
                for cs in halves:
                    nc.scalar.activation(sb_fi[:, cs], ps_fi[:, cs], AF.Sigmoid)
                    nc.scalar.activation(sb_o[:, cs], ps_og[0:H1, cs], AF.Sigmoid)
                    nc.vector.tensor_mul(prods[:, cs], sb_fi[:, cs], X1[:, cs])
                    # realign hi half to base partition 0 (4x copy), then add
                    pt = sp.tile([H1, Bc], f16, tag="pt")
                    nc.vector.tensor_copy(pt[:, cs], prods[H1:128, cs])
                    nc.vector.tensor_add(X1[0:H1, cs], prods[0:H1, cs], pt[:, cs])
                    hsl = slice(base + Bc + cs.start, base + Bc + cs.stop)
                    nc.vector.tensor_mul(cat1[0:H1, hsl], sb_o[:, cs], X1[0:H1, cs])
            if 1 <= s_mm <= T:
                lstm2_sig(s_mm)  # end of ACT queue + Pool fc2

        # ================= phase 2: LSTM3 + dense ========================
        # Dense outputs for 3 consecutive steps stack into one [88, Bc] psum
        # region (matmul psum partition offsets must be 0/32/64), drained by
        # ONE activation + ONE DMA per group. Groups ping-pong psum buffers.
        GRP = 3
        ps_d = None
        for t in range(T + 1):
            cur3, nxt3 = cat3[t % 2], cat3[(t + 1) % 2]
            if t < T:
                ps_og3 = pp.tile([128, Bc], f32, tag="pB")
                for cs in halves:
                    nc.tensor.matmul(ps_og3[:, cs], w_og3, cur3[:, cs], start=True, stop=True)
                ps_fi3 = pp.tile([128, Bc], f32, tag="pA")
                for cs in halves:
                    nc.tensor.matmul(ps_fi3[:, cs], w_fi3, cur3[:, cs], start=True, stop=True)
            if t >= 1:
                # dense on h3_t (cur3 lo 65 rows = [h3_t; 1]); bias via ones-row
                j = (t - 1) % GRP
                g = (t - 1) // GRP
                if j == 0:
                    ps_d = pp.tile([128, Bc], f32, tag=f"pC{g % 2}")
                for cs in halves:
                    nc.tensor.matmul(
                        ps_d[32 * j : 32 * j + F, cs], w_d, cur3[0 : H3 + 1, cs],
                        start=True, stop=True,
                    )
            if t < T:
                # relu g on DVE (off the ACT chain), per-half cell chains
                sb_fi3 = sp.tile([128, Bc], f16, tag="sb_fi")
                sb_o3 = sp.tile([H3, Bc], f16, tag="sb_o")
                prods3 = sp.tile([128, Bc], f16, tag="prods")
                for cs in halves:
                    nc.vector.tensor_scalar_max(X3[H3:128, cs], ps_og3[H3:128, cs], 0.0)
                for cs in halves:
                    nc.scalar.activation(sb_fi3[:, cs], ps_fi3[:, cs], AF.Sigmoid)
                    nc.scalar.activation(sb_o3[:, cs], ps_og3[0:H3, cs], AF.Sigmoid)
                    nc.vector.tensor_mul(prods3[:, cs], sb_fi3[:, cs], X3[:, cs])
                    pt3 = sp.tile([H3, Bc], f16, tag="pt")
                    nc.vector.tensor_copy(pt3[:, cs], prods3[H3:128, cs])
                    nc.vector.tensor_add(X3[0:H3, cs], prods3[0:H3, cs], pt3[:, cs])
                    nc.vector.tensor_mul(nxt3[0:H3, cs], sb_o3[:, cs], X3[0:H3, cs])
            if t >= 1 and ((t - 1) % GRP == GRP - 1 or t == T):
                g = (t - 1) // GRP
                rows = 32 * ((t - 1) % GRP) + F
                yt = op.tile([32 * GRP, Bc], f16, tag="yt")
                nc.scalar.activation(yt[0:rows], ps_d[0:rows], AF.Copy)
                nc.sync.dma_start(out=yT_d[g], in_=yt)

    return nc


def _prep_inputs(inputs):
    """Host-side: shard batch, transpose x, pack weights. Returns in_maps."""
    x = np.asarray(inputs["x"], np.float32)
    W1, U1, b1 = (np.asarray(inputs[k], np.float32) for k in ("W1", "U1", "b1"))
    W2, U2, b2 = (np.asarray(inputs[k], np.float32) for k in ("W2", "U2", "b2"))
    W3, U3, b3 = (np.asarray(inputs[k], np.float32) for k in ("W3", "U3", "b3"))
    Wd, bd = (np.asarray(inputs[k], np.float32) for k in ("Wd", "bd"))

    # Reference gate column order is (i, f, g, o) in 4H blocks.
    def cols(H, *gates):
        idx = {"i": 0, "f": 1, "g": 2, "o": 3}
        return np.concatenate([np.arange(idx[g] * H, (idx[g] + 1) * H) for g in gates])

    uw1 = np.concatenate([U1, W1, b1[None, :]], axis=0)  # [89, 256]
    w_fi1 = uw1[:, cols(H1, "f", "i")].astype(np.float16)
    w_og1 = uw1[:, cols(H1, "o", "g")].astype(np.float16)

    perm2 = cols(LAT, "f", "i", "o", "g")
    ub2 = np.concatenate([U2, b2[None, :]], axis=0)  # [33, 128]
    w_u2 = ub2[:, perm2].astype(np.float16)
    w_w2 = W2[:, perm2].astype(np.float16)

    # rhs for LSTM3 is [h3; 1; pad; z] so rows are [U3; b3; 0-pad; W3]
    uw3 = np.concatenate(
        [U3, b3[None, :], np.zeros((31, 4 * H3), np.float32), W3], axis=0
    )  # [128, 256]
    w_fi3 = uw3[:, cols(H3, "f", "i")].astype(np.float16)
    w_og3 = uw3[:, cols(H3, "o", "g")].astype(np.float16)

    w_d = np.concatenate([Wd, bd[None, :]], axis=0).astype(np.float16)  # [65, 24]

    in_maps = []
    for c in range(NCORES):
        xc = x[c * Bc : (c + 1) * Bc]  # [Bc, T, F]
        xt = xc.transpose(1, 2, 0).astype(np.float16)  # [T, F, Bc]
        xt = np.concatenate([xt, np.ones((T, 1, Bc), np.float16)], axis=1)
        in_maps.append(
            {
                "xT": np.ascontiguousarray(xt),
                "w_fi1": w_fi1,
                "w_og1": w_og1,
                "w_u2": w_u2,
                "w_w2": w_w2,
                "w_fi3": w_fi3,
                "w_og3": w_og3,
                "w_d": w_d,
            }
        )
    return in_maps


def _make_runner(nc):
    """Compile nc once into a sharded 8-core jit; returns run(in_maps)->results.

    Mirrors bass2jax.run_bass_via_pjrt but caches the compiled executable so
    repeated calls only pay device dispatch.
    """
    import jax
    from jax.sharding import Mesh, PartitionSpec
    from jax.experimental.shard_map import shard_map
    from concourse import bass2jax, mybir as _mb

    bass2jax.install_neuronx_cc_hook()

    partition_name = nc.partition_id_tensor.name if nc.partition_id_tensor else None
    in_names, out_names, out_avals, zero_outs = [], [], [], []
    for alloc in nc.m.functions[0].allocations:
        if not isinstance(alloc, _mb.MemoryLocationSet):
            continue
        name = alloc.memorylocations[0].name
        if alloc.kind == "ExternalInput":
            if name != partition_name:
                in_names.append(name)
        elif alloc.kind == "ExternalOutput":
            out_names.append(name)
            shape = tuple(alloc.tensor_shape)
            dtype = _mb.dt.np(alloc.dtype)
            out_avals.append(jax.core.ShapedArray(shape, dtype))
            zero_outs.append(np.zeros(shape, dtype))
    n_params = len(in_names)
    n_outs = len(out_avals)
    all_in_names = list(in_names) + list(out_names)
    if partition_name is not None:
        all_in_names.append(partition_name)

    def _bind(ins, outs):
        operands = list(ins) + list(outs)
        if partition_name is not None:
            operands.append(bass2jax.partition_id_tensor())
        return bass2jax._bass_exec_p.bind(
            *operands,
            out_avals=tuple(out_avals),
            in_names=tuple(all_in_names),
            out_names=tuple(out_names),
            lowering_input_output_aliases=(),
            sim_require_finite=True,
            sim_require_nnan=True,
            nc=nc,
        )

    def _body(*args):
        return tuple(_bind(args[:n_params], args[n_params:]))

    devices = jax.devices()[:NCORES]
    mesh = Mesh(np.asarray(devices), ("core",))
    in_specs = (PartitionSpec("core"),) * (n_params + n_outs)
    out_specs = (PartitionSpec("core"),) * len(out_names)
    sharded = jax.jit(
        shard_map(
            _body, mesh=mesh, in_specs=in_specs, out_specs=out_specs, check_rep=False
        ),
        keep_unused=True,
    )

    def run(in_maps, timing_reps=0):
        import time as _time
        from jax.sharding import NamedSharding

        sh = NamedSharding(mesh, PartitionSpec("core"))
        concat_in = [
            jax.device_put(
                np.concatenate([np.asarray(m[name]) for m in in_maps], axis=0), sh
            )
            for name in in_names
        ]
        concat_zeros = [
            jax.device_put(np.zeros((NCORES * z.shape[0], *z.shape[1:]), z.dtype), sh)
            for z in zero_outs
        ]
        out_arrs = jax.block_until_ready(sharded(*concat_in, *concat_zeros))
        times = []
        if timing_reps:
            for _ in range(timing_reps):
                t0 = _time.perf_counter()
                jax.block_until_ready(sharded(*concat_in, *concat_zeros))
                times.append(_time.perf_counter() - t0)
        results = [
            {
                name: np.asarray(out_arrs[i]).reshape(NCORES, *out_avals[i].shape)[c]
                for i, name in enumerate(out_names)
            }
            for c in range(NCORES)
        ]
        return results, times

    return run


def _get_runner(repeat=1):
    key = f"runner{repeat}"
    if key not in _CACHE:
        _CACHE[key] = _make_runner(_wrap_to_json(_build_nc(repeat=repeat)))
    return _CACHE[key]


def _run(inputs, trace=False, timing_reps=0):
    in_maps = _prep_inputs(inputs)
    results, times = _get_runner(1)(in_maps, timing_reps=timing_reps)
    y = np.empty((B, T, F), np.float32)
    for c in range(NCORES):
        yt = results[c]["yT"]  # [NG, 96, Bc] f16 -> steps in 32-row blocks
        yt = yt.reshape(-1, 32, Bc)[0:T, 0:F]
        y[c * Bc : (c + 1) * Bc] = yt.transpose(2, 0, 1).astype(np.float32)
    return y, times


def kernel(**inputs):
    y, _ = _run(inputs)
    return y
